# revision 25
# baseline (speedup 1.0000x reference)
"""Block-sparse matmul kernel for Trainium2 (8 NeuronCores, SPMD).

Problem: out = relu(x @ W_sparse + bias)
  x      [1024, 4096] f32
  kernel [4096, 32, 32] f32   (active 32x32 blocks)
  bias   [4096] f32
  ci, co [4096] int32         (block-row / block-col of each active block)
  out    [1024, 4096] f32

Strategy (v1, dense): scatter blocks into a dense [4096, 4096] weight
matrix on the host, cast x/W to bf16, and run a dense GEMM sharded
2-way over batch x 4-way over output columns (8 cores).  Each core
computes outT = W_slab.T @ x_half.T in [out, batch] orientation so
bias becomes a per-partition scalar for the ScalarE activation
(fused bias + relu straight out of PSUM).  The Bass program is
identical on all cores; only the data differs (SPMD-safe).
"""

import numpy as np
import ml_dtypes

import concourse.bacc as bacc
import concourse.bass as bass
import concourse.mybir as mybir
import concourse.tile as tile
from concourse.bass_utils import run_bass_kernel_spmd

BS = 32
N_IN = 4096
N_OUT = 4096
BATCH = 1024
N_CORES = 8

# sharding grid: 4 output-column quarters x 2 batch halves
CO_SHARDS = 4
B_SHARDS = 2
M_PER_CORE = N_OUT // CO_SHARDS          # 1024 output cols per core
B_PER_CORE = BATCH // B_SHARDS           # 512 batch rows per core
N_MTILES = M_PER_CORE // 128             # 8
N_KTILES = N_IN // 128                   # 32

BF16 = mybir.dt.bfloat16
F32 = mybir.dt.float32

_CACHE = {}


def _build_program():
    """Dense GEMM program, one core's share: outT[m,p,b] = relu(sum_k
    W[k,128m+p] * x[b,k] + bias[128m+p]).  Identical on all cores."""
    nc = bacc.Bacc(trn_type="TRN2")

    xT_d = nc.dram_tensor("xT", [128, N_KTILES * B_PER_CORE], BF16,
                          kind="ExternalInput")
    # wK[p, k*1024 + m*128 + c] = Wdense[128k+p, 128m+c] (per-core slab)
    wK_d = nc.dram_tensor("wK", [128, N_KTILES * N_MTILES * 128], BF16,
                          kind="ExternalInput")
    bias_d = nc.dram_tensor("biasv", [128, N_MTILES], F32,
                            kind="ExternalInput")
    outT_d = nc.dram_tensor("outT", [N_MTILES, 128, B_PER_CORE], F32,
                            kind="ExternalOutput")

    MCOLS = N_MTILES * 128  # 1024 W cols per k-tile

    with tile.TileContext(nc) as tc:
        with (
            tc.tile_pool(name="xp", bufs=1) as xp,
            tc.tile_pool(name="wp", bufs=1) as wp,
            tc.tile_pool(name="bp", bufs=1) as bp,
            tc.tile_pool(name="op", bufs=8) as op,
            tc.tile_pool(name="ps", bufs=8, space="PSUM") as ps,
            tc.tile_pool(name="wu", bufs=1) as wu,
        ):
            accs = [ps.tile([128, B_PER_CORE], F32, tag="acc",
                            name=f"acc{m}")
                    for m in range(N_MTILES)]

            # --- HAM warmup: keep PE busy while the first DMAs land so
            # the real matmul stream starts at K=8/8 (2.4 GHz).
            wut = wu.tile([128, 128], BF16)
            nc.vector.memset(wut[:], 0.0)
            for _ in range(22):
                nc.tensor.matmul(accs[N_MTILES - 1][:, 0:128],
                                 wut[:], wut[:], start=True, stop=True)

            # x and W fully resident in SBUF, streamed in k-order chunks.
            # Early chunks are small (fast arrival, bridges the warmup);
            # later chunks use long per-partition rows to amortize the
            # ~0.6us/row DMA overhead.  Sync frees up first after the
            # preamble, so it carries the critical path; gpsimd/scalar
            # carry mid-stream W.
            xt = xp.tile([128, N_KTILES * B_PER_CORE], BF16)
            wt = wp.tile([128, N_KTILES * MCOLS], BF16)

            def xs(a, b):
                return slice(a * B_PER_CORE, b * B_PER_CORE)

            def wsl(a, b):
                return slice(a * MCOLS, b * MCOLS)

            # One sync HWDGE queue sustains ~390 GB/s regardless of chunk
            # size (measured); multiple queues contend and run slower.
            # So: everything on sync, in exact consumption order, finer
            # chunks early so the stream can start ~10us in.
            kb = [0, 1, 2, 3, 4, 5, 6, 8, 10, 12, 16, 20, 24, 28, 32]
            for j in range(len(kb) - 1):
                a, b = kb[j], kb[j + 1]
                nc.sync.dma_start(xt[:, xs(a, b)], xT_d[:, xs(a, b)])
                nc.sync.dma_start(wt[:, wsl(a, b)], wK_d[:, wsl(a, b)])

            bv = bp.tile([128, N_MTILES], F32)
            nc.sync.dma_start(bv[:], bias_d[:])

            def mm(m, k):
                nc.tensor.matmul(
                    accs[m][:],
                    wt[:, k * MCOLS + m * 128: k * MCOLS + (m + 1) * 128],
                    xt[:, k * B_PER_CORE:(k + 1) * B_PER_CORE],
                    start=(k == 0),
                    stop=(k == N_KTILES - 1),
                )

            # Phase 1, k-outer / m-inner: all 8 PSUM banks accumulate
            # concurrently; step k consumes only x[k]+W[k] (384KiB).
            KSPLIT = N_KTILES - 8
            for k in range(KSPLIT):
                for m in range(N_MTILES):
                    mm(m, k)

            # Phase 2, m-outer: finish each m's last k-tiles, then evict
            # while the next m still matmuls.  Evictions alternate
            # ScalarE / VectorE; out-DMAs overlap the stream tail.
            for m in range(N_MTILES):
                for k in range(KSPLIT, N_KTILES):
                    mm(m, k)
                ot = op.tile([128, B_PER_CORE], F32, tag="o")
                if m % 2 == 0:
                    nc.scalar.activation(ot[:], accs[m][:],
                                         mybir.ActivationFunctionType.Relu,
                                         bias=bv[:, m:m + 1])
                else:
                    nc.vector.tensor_scalar(ot[:], accs[m][:],
                                            bv[:, m:m + 1], 0.0,
                                            mybir.AluOpType.add,
                                            mybir.AluOpType.max)
                nc.sync.dma_start(outT_d[m], ot[:])

    nc.compile()
    return nc


def _dense_weight(kernel_blocks, ci, co):
    """Scatter [N_BLK,32,32] blocks into dense [N_IN, N_OUT] (duplicates sum)."""
    nbr, nbc = N_IN // BS, N_OUT // BS
    wd4 = np.zeros((nbr, nbc, BS, BS), np.float32)
    np.add.at(wd4, (ci.astype(np.int64), co.astype(np.int64)),
              kernel_blocks.astype(np.float32))
    return wd4.transpose(0, 2, 1, 3).reshape(N_IN, N_OUT)


def _prep_inputs(x, kernel_blocks, bias, ci, co):
    x = np.asarray(x, np.float32)
    bias = np.asarray(bias, np.float32)
    ci = np.asarray(ci)
    co = np.asarray(co)
    wd = _dense_weight(np.asarray(kernel_blocks), ci, co)

    x_bf = x.astype(ml_dtypes.bfloat16)
    wd_bf = wd.astype(ml_dtypes.bfloat16)

    in_maps = []
    for c in range(N_CORES):
        q, h = divmod(c, B_SHARDS)
        xs = x_bf[h * B_PER_CORE:(h + 1) * B_PER_CORE]      # [512, 4096]
        # xT[p, k*512+b] = xs[b, 128k+p]
        xT = np.ascontiguousarray(
            xs.reshape(B_PER_CORE, N_KTILES, 128).transpose(2, 1, 0)
            .reshape(128, N_KTILES * B_PER_CORE))
        ws = wd_bf[:, q * M_PER_CORE:(q + 1) * M_PER_CORE]  # [4096, 1024]
        # wK[p, k*1024 + m*128 + cc] = ws[128k+p, 128m+cc]
        wK = np.ascontiguousarray(
            ws.reshape(N_KTILES, 128, N_MTILES * 128).transpose(1, 0, 2)
            .reshape(128, N_KTILES * N_MTILES * 128))
        bs = bias[q * M_PER_CORE:(q + 1) * M_PER_CORE]
        biasv = np.ascontiguousarray(bs.reshape(N_MTILES, 128).T)
        in_maps.append({"xT": xT, "wK": wK, "biasv": biasv})
    return in_maps


def _assemble(results):
    out = np.empty((BATCH, N_OUT), np.float32)
    for c in range(N_CORES):
        q, h = divmod(c, B_SHARDS)
        o = results[c]["outT"]  # [8, 128, 512] = [m, p, b]
        out[h * B_PER_CORE:(h + 1) * B_PER_CORE,
            q * M_PER_CORE:(q + 1) * M_PER_CORE] = (
            o.transpose(2, 0, 1).reshape(B_PER_CORE, M_PER_CORE))
    return out


def run(x, kernel, bias, ci, co, trace=False):
    if "nc" not in _CACHE:
        _CACHE["nc"] = _build_program()
    nc = _CACHE["nc"]
    in_maps = _prep_inputs(x, kernel, bias, ci, co)
    last_err = None
    for attempt in range(3):
        try:
            res = run_bass_kernel_spmd(nc, in_maps,
                                       core_ids=list(range(N_CORES)),
                                       trace=trace)
            return _assemble(res.results), res
        except Exception as e:  # transient NRT_EXEC_UNIT_UNRECOVERABLE
            last_err = e
            import time
            time.sleep(2.0)
    raise last_err


def kernel(x, kernel, bias, ci, co):
    out, _ = run(x, kernel, bias, ci, co, trace=False)
    return out


# revision 26
# speedup vs baseline: 1.0267x; 1.0267x over previous
"""Block-sparse matmul kernel for Trainium2 (8 NeuronCores, SPMD).

Problem: out = relu(x @ W_sparse + bias)
  x      [1024, 4096] f32
  kernel [4096, 32, 32] f32   (active 32x32 blocks)
  bias   [4096] f32
  ci, co [4096] int32         (block-row / block-col of each active block)
  out    [1024, 4096] f32

Strategy (v1, dense): scatter blocks into a dense [4096, 4096] weight
matrix on the host, cast x/W to bf16, and run a dense GEMM sharded
2-way over batch x 4-way over output columns (8 cores).  Each core
computes outT = W_slab.T @ x_half.T in [out, batch] orientation so
bias becomes a per-partition scalar for the ScalarE activation
(fused bias + relu straight out of PSUM).  The Bass program is
identical on all cores; only the data differs (SPMD-safe).
"""

import numpy as np
import ml_dtypes

import concourse.bacc as bacc
import concourse.bass as bass
import concourse.mybir as mybir
import concourse.tile as tile
from concourse.bass_utils import run_bass_kernel_spmd

BS = 32
N_IN = 4096
N_OUT = 4096
BATCH = 1024
N_CORES = 8

# sharding grid: 4 output-column quarters x 2 batch halves
CO_SHARDS = 4
B_SHARDS = 2
M_PER_CORE = N_OUT // CO_SHARDS          # 1024 output cols per core
B_PER_CORE = BATCH // B_SHARDS           # 512 batch rows per core
N_MTILES = M_PER_CORE // 128             # 8
N_KTILES = N_IN // 128                   # 32

BF16 = mybir.dt.bfloat16
F32 = mybir.dt.float32

_CACHE = {}


def _build_program():
    """Dense GEMM program, one core's share: outT[m,p,b] = relu(sum_k
    W[k,128m+p] * x[b,k] + bias[128m+p]).  Identical on all cores."""
    nc = bacc.Bacc(trn_type="TRN2")

    xT_d = nc.dram_tensor("xT", [128, N_KTILES * B_PER_CORE], BF16,
                          kind="ExternalInput")
    # wK[p, k*1024 + m*128 + c] = Wdense[128k+p, 128m+c] (per-core slab)
    wK_d = nc.dram_tensor("wK", [128, N_KTILES * N_MTILES * 128], BF16,
                          kind="ExternalInput")
    bias_d = nc.dram_tensor("biasv", [128, N_MTILES], F32,
                            kind="ExternalInput")
    outT_d = nc.dram_tensor("outT", [N_MTILES, 128, B_PER_CORE], F32,
                            kind="ExternalOutput")

    MCOLS = N_MTILES * 128  # 1024 W cols per k-tile

    with tile.TileContext(nc) as tc:
        with (
            tc.tile_pool(name="xp", bufs=1) as xp,
            tc.tile_pool(name="wp", bufs=1) as wp,
            tc.tile_pool(name="bp", bufs=1) as bp,
            tc.tile_pool(name="op", bufs=8) as op,
            tc.tile_pool(name="ps", bufs=8, space="PSUM") as ps,
            tc.tile_pool(name="wu", bufs=1) as wu,
        ):
            accs = [ps.tile([128, B_PER_CORE], F32, tag="acc",
                            name=f"acc{m}")
                    for m in range(N_MTILES)]

            # --- HAM warmup: keep PE busy while the first DMAs land so
            # the real matmul stream starts at K=8/8 (2.4 GHz).
            wut = wu.tile([128, 128], BF16)
            nc.vector.memset(wut[:], 0.0)
            for _ in range(40):
                nc.tensor.matmul(accs[N_MTILES - 1][:, 0:128],
                                 wut[:], wut[:], start=True, stop=True)

            # x and W fully resident in SBUF, streamed in k-order chunks.
            # Early chunks are small (fast arrival, bridges the warmup);
            # later chunks use long per-partition rows to amortize the
            # ~0.6us/row DMA overhead.  Sync frees up first after the
            # preamble, so it carries the critical path; gpsimd/scalar
            # carry mid-stream W.
            xt = xp.tile([128, N_KTILES * B_PER_CORE], BF16)
            wt = wp.tile([128, N_KTILES * MCOLS], BF16)

            def xs(a, b):
                return slice(a * B_PER_CORE, b * B_PER_CORE)

            def wsl(a, b):
                return slice(a * MCOLS, b * MCOLS)

            # One sync HWDGE queue sustains ~390 GB/s regardless of chunk
            # size (measured); multiple queues contend and run slower.
            # So: everything on sync, in exact consumption order, finer
            # chunks early so the stream can start ~10us in.
            kb = [0, 1, 2, 3, 4, 5, 6, 8, 10, 12, 16, 20, 24, 28, 32]
            for j in range(len(kb) - 1):
                a, b = kb[j], kb[j + 1]
                nc.sync.dma_start(xt[:, xs(a, b)], xT_d[:, xs(a, b)])
                nc.sync.dma_start(wt[:, wsl(a, b)], wK_d[:, wsl(a, b)])

            bv = bp.tile([128, N_MTILES], F32)
            nc.sync.dma_start(bv[:], bias_d[:])

            def mm(m, k):
                nc.tensor.matmul(
                    accs[m][:],
                    wt[:, k * MCOLS + m * 128: k * MCOLS + (m + 1) * 128],
                    xt[:, k * B_PER_CORE:(k + 1) * B_PER_CORE],
                    start=(k == 0),
                    stop=(k == N_KTILES - 1),
                )

            # Phase 1, k-outer / m-inner: all 8 PSUM banks accumulate
            # concurrently; step k consumes only x[k]+W[k] (384KiB).
            KSPLIT = N_KTILES - 8
            for k in range(KSPLIT):
                for m in range(N_MTILES):
                    mm(m, k)

            # Phase 2, m-outer: finish each m's last k-tiles, then evict
            # while the next m still matmuls.  Evictions alternate
            # ScalarE / VectorE; out-DMAs overlap the stream tail.
            for m in range(N_MTILES):
                for k in range(KSPLIT, N_KTILES):
                    mm(m, k)
                ot = op.tile([128, B_PER_CORE], F32, tag="o")
                if m % 2 == 0:
                    nc.scalar.activation(ot[:], accs[m][:],
                                         mybir.ActivationFunctionType.Relu,
                                         bias=bv[:, m:m + 1])
                else:
                    nc.vector.tensor_scalar(ot[:], accs[m][:],
                                            bv[:, m:m + 1], 0.0,
                                            mybir.AluOpType.add,
                                            mybir.AluOpType.max)
                nc.sync.dma_start(outT_d[m], ot[:])

    nc.compile()
    return nc


def _dense_weight(kernel_blocks, ci, co):
    """Scatter [N_BLK,32,32] blocks into dense [N_IN, N_OUT] (duplicates sum)."""
    nbr, nbc = N_IN // BS, N_OUT // BS
    wd4 = np.zeros((nbr, nbc, BS, BS), np.float32)
    np.add.at(wd4, (ci.astype(np.int64), co.astype(np.int64)),
              kernel_blocks.astype(np.float32))
    return wd4.transpose(0, 2, 1, 3).reshape(N_IN, N_OUT)


def _prep_inputs(x, kernel_blocks, bias, ci, co):
    x = np.asarray(x, np.float32)
    bias = np.asarray(bias, np.float32)
    ci = np.asarray(ci)
    co = np.asarray(co)
    wd = _dense_weight(np.asarray(kernel_blocks), ci, co)

    x_bf = x.astype(ml_dtypes.bfloat16)
    wd_bf = wd.astype(ml_dtypes.bfloat16)

    in_maps = []
    for c in range(N_CORES):
        q, h = divmod(c, B_SHARDS)
        xs = x_bf[h * B_PER_CORE:(h + 1) * B_PER_CORE]      # [512, 4096]
        # xT[p, k*512+b] = xs[b, 128k+p]
        xT = np.ascontiguousarray(
            xs.reshape(B_PER_CORE, N_KTILES, 128).transpose(2, 1, 0)
            .reshape(128, N_KTILES * B_PER_CORE))
        ws = wd_bf[:, q * M_PER_CORE:(q + 1) * M_PER_CORE]  # [4096, 1024]
        # wK[p, k*1024 + m*128 + cc] = ws[128k+p, 128m+cc]
        wK = np.ascontiguousarray(
            ws.reshape(N_KTILES, 128, N_MTILES * 128).transpose(1, 0, 2)
            .reshape(128, N_KTILES * N_MTILES * 128))
        bs = bias[q * M_PER_CORE:(q + 1) * M_PER_CORE]
        biasv = np.ascontiguousarray(bs.reshape(N_MTILES, 128).T)
        in_maps.append({"xT": xT, "wK": wK, "biasv": biasv})
    return in_maps


def _assemble(results):
    out = np.empty((BATCH, N_OUT), np.float32)
    for c in range(N_CORES):
        q, h = divmod(c, B_SHARDS)
        o = results[c]["outT"]  # [8, 128, 512] = [m, p, b]
        out[h * B_PER_CORE:(h + 1) * B_PER_CORE,
            q * M_PER_CORE:(q + 1) * M_PER_CORE] = (
            o.transpose(2, 0, 1).reshape(B_PER_CORE, M_PER_CORE))
    return out


def run(x, kernel, bias, ci, co, trace=False):
    if "nc" not in _CACHE:
        _CACHE["nc"] = _build_program()
    nc = _CACHE["nc"]
    in_maps = _prep_inputs(x, kernel, bias, ci, co)
    last_err = None
    for attempt in range(3):
        try:
            res = run_bass_kernel_spmd(nc, in_maps,
                                       core_ids=list(range(N_CORES)),
                                       trace=trace)
            return _assemble(res.results), res
        except Exception as e:  # transient NRT_EXEC_UNIT_UNRECOVERABLE
            last_err = e
            import time
            time.sleep(2.0)
    raise last_err


def kernel(x, kernel, bias, ci, co):
    out, _ = run(x, kernel, bias, ci, co, trace=False)
    return out
